# revision 8
# baseline (speedup 1.0000x reference)
"""Trainium2 Bass kernel for DigitConvolutionalModel (dense_cnn).

Network: x[B,784] -> 3x3 valid conv (1 ch) -> flatten -> MLP
         (676->200 relu, 200->200 relu, 200->200 relu, 200->10).
Conv+W1 fold into W1p = C @ W1 [784, 200] (conv is linear, feeds W1 with no
nonlinearity between), so the network is a plain 4-layer MLP. Sharding: pure
data parallel over 8 cores (batch 65536 -> 8192/core); activations stay
feature-major on device; host packs/unpacks.

Speed scheme (vs the f32r baseline at ~80.4us): layers in FP8L run their
matmuls in fp8 e4m3 with perf_mode=DoubleRow, which streams the moving tensor
at 0.5 cycles/row AND contracts TWO independent (lhsT, rhs) k-tiles per
instruction (4x f32r MAC throughput). fp8's 3 mantissa bits alone would blow
the 2e-2 error gate, so each fp8 layer accumulates THREE error-compensation
terms into one PSUM group:

    x@W ~= x_hi@W_hi + x_hi@W_lo + x_lo@W_hi,   _hi = e4m3(v*S),
                                                _lo = e4m3(v*S - _hi)

(dropped x_lo@W_lo term is O(2^-8) relative). Host-simulated end-to-end
rel err: all-fp8 2.0e-3, L1-only 1.4e-3-ish (vs 3.7e-4 f32r baseline).

Per-group matmul cost (cycles, NB=512 moving columns):
    L1: f32r 14x512=7168 | fp8-T3 10 DRx2 chunks x256 = 5120
    L2/L3: f32r 4x512    | fp8-T3 3 DRx2x256 = 1536
    L4: f32r 2x512       | fp8-T3 3 DRx256 = 768

All hidden-feature splits are chunks of (100,100) so one DoubleRow pair spans
both chunks at the same SBUF partitions. When layer l+1 is fp8, layer l's
output is split on-device: ACT writes t = relu(ps*scale + b*SH) f32, DVE
casts hi = e4m3(t), Pool computes lo = e4m3(t - hi) - one pass per engine.
L1's x_hi/x_lo come pre-split from the host. Scales are powers of two folded
into the consuming ACT's scale and the packed biases.
"""

import numpy as np
import ml_dtypes

import concourse.bacc as bacc
import concourse.mybir as mybir
import concourse.tile as tile
from concourse.bass_utils import run_bass_kernel_spmd

B = 65536
IMG = 28
KW = 3
HID = 200
OUT = 10
K1 = IMG * IMG  # 784

N_CORES = 8
BC = B // N_CORES  # 8192
NB = 512
NG = BC // NB  # 16
HC = 100  # hidden feature chunk (2 chunks of 100)

F32 = mybir.dt.float32
F32R = mybir.dt.float32r
FP8 = mybir.dt.float8e4
E4 = ml_dtypes.float8_e4m3
DRMODE = mybir.MatmulPerfMode.DoubleRow
AOP = mybir.AluOpType

# which layers run fp8-T3 DoubleRow; layer l>=2 in FP8L makes layer l-1 emit
# hi/lo fp8 activations (device-side split via ACT+DVE+Pool).
import os as _os
FP8L = frozenset(int(c) for c in _os.environ.get("BASS_FP8L", "1"))

# fp8 x layout: 6 full 128-row chunks + 16-row tail; slots 0-5 hi, 6-11 lo,
# 12/13 = tail block (rows 0-15 hi-tail, 16-31 lo-tail, rest zero), slot 13
# a copy of 12 so the tail DoubleRow pair has a real second AP step.
XCH = 6
XTAIL = K1 - XCH * 128  # 16
NSLOT = 14
SLOTW = 112   # fp8 weight slot stride (bytes): must be even + 16B-aligned
SLOTW4 = 16   # L4 slot stride

SX = 16.0   # x scale (|x|max ~5.5 -> 88, e4m3 max 240)
SH = 4.0    # fp8 hidden activation scale (h absmax ~18 -> 72)
SW = {1: 32.0, 2: 64.0, 3: 64.0, 4: 64.0}  # weight scales

F32R_K = 112  # f32r L1 k-chunk (7 chunks)
NK1 = K1 // F32R_K

# ---- f32 weight blob column layout ----
L1R_COL = 0                      # 7k x 2c blocks [112, 100]
W2_COL = L1R_COL + NK1 * HID     # 2k x 2c blocks [100, 100]
W3_COL = W2_COL + 2 * HID
W4_COL = W3_COL + 2 * HID        # 2k blocks [100, 10]
B_COL = W4_COL + 2 * OUT         # b1(2 cols) b2(2) b3(2)
B4_COL = B_COL + 6               # b4 [10, 1]
WC = B4_COL + 1

_cache: dict = {}


def _f8_offsets(fp8l):
    off, cols = {}, 0
    if 1 in fp8l:
        off[1] = cols
        cols += 2 * NSLOT * SLOTW
    for l in (2, 3):
        if l in fp8l:
            off[l] = cols
            cols += 2 * 4 * SLOTW
    if 4 in fp8l:
        off[4] = cols
        cols += 4 * SLOTW4
    return off, max(cols, 4)


def _scales(fp8l):
    """per-layer (act_scale, out_sh, ps_scale)."""
    out_sh = {l: (SH if (l + 1) in fp8l else 1.0) for l in (1, 2, 3)}
    ins = {1: SX, 2: out_sh[1], 3: out_sh[2], 4: out_sh[3]}
    ps = {l: (ins[l] * SW[l] if l in fp8l else 1.0) for l in (1, 2, 3, 4)}
    return out_sh, ps


def _build(mode: str = "fp8", repeats: int = 1, xbufs: int = 3, hbufs: int = 2,
           obufs: int = 2):
    fp8l = FP8L if mode == "fp8" else frozenset()
    F8_OFF, F8C = _f8_offsets(fp8l)
    OUT_SH, PS_SCALE = _scales(fp8l)
    relu = mybir.ActivationFunctionType.Relu

    nc = bacc.Bacc("TRN2", target_bir_lowering=False, debug=False)

    if 1 in fp8l:
        xh = nc.dram_tensor("xh", [NG * 128, NSLOT * NB], FP8, kind="ExternalInput")
    else:
        xh = nc.dram_tensor("xh", [NG * F32R_K, NK1 * NB], F32R, kind="ExternalInput")
    wb = nc.dram_tensor("wb", [128, WC], F32R, kind="ExternalInput")
    wf = nc.dram_tensor("wf", [128, F8C], FP8, kind="ExternalInput")
    outT = nc.dram_tensor("outT", [OUT, BC], F32, kind="ExternalOutput")

    with tile.TileContext(nc) as tc:
        with (
            tc.tile_pool(name="wpool", bufs=1) as wpool,
            tc.tile_pool(name="xpool", bufs=xbufs) as xpool,
            tc.tile_pool(name="hpool", bufs=hbufs) as hpool,
            tc.tile_pool(name="opool", bufs=obufs) as opool,
            tc.tile_pool(name="psum", bufs=1, space="PSUM") as psum,
        ):
            wt = wpool.tile([128, WC], F32R, tag="wt")
            nc.scalar.dma_start(out=wt, in_=wb.ap())
            wft = wpool.tile([128, F8C], FP8, tag="wft")
            nc.gpsimd.dma_start(out=wft, in_=wf.ap())

            def bias(idx, hsz):  # f32 bias column [hsz, 1]
                return wt[0:hsz, B_COL + idx : B_COL + idx + 1].bitcast(F32)

            def w1f8(c):  # [128, NSLOT, 100] L1 fp8 slots, M-chunk c
                o = F8_OFF[1] + c * NSLOT * SLOTW
                return wft[:, o : o + NSLOT * SLOTW].rearrange(
                    "p (s m) -> p s m", s=NSLOT)[:, :, 0:HC]

            def wlf8(l, c):  # [100, 4, sz] layer-l fp8 slots (Wh0 Wh1 Wl0 Wl1)
                sz, sw = (OUT, SLOTW4) if l == 4 else (HC, SLOTW)
                o = F8_OFF[l] + c * 4 * sw
                return wft[0:HC, o : o + 4 * sw].rearrange(
                    "p (s m) -> p s m", s=4)[:, :, 0:sz]

            # ---------- matmul chains ----------
            def l1_fp8(xg, c):
                ps = psum.tile([HC, NB], F32, tag=f"ps1_{c}", name="ps")
                wfc = w1f8(c)
                pairs = ((0, 0), (2, 2), (4, 4),     # x_hi @ W_hi
                         (0, 6), (2, 8), (4, 10),    # x_hi @ W_lo
                         (6, 0), (8, 2), (10, 4))    # x_lo @ W_hi
                for j, (xs, ws) in enumerate(pairs):
                    nc.tensor.matmul(
                        ps, wfc[:, ws : ws + 2, :], xg[:, xs : xs + 2, :],
                        start=(j == 0), stop=False, perf_mode=DRMODE,
                    )
                nc.tensor.matmul(  # tails: all three terms in one 32-row pair
                    ps, wfc[0:32, 12:14, :], xg[0:32, 12:14, :],
                    start=False, stop=True, perf_mode=DRMODE,
                )
                return ps

            def l1_f32r(xg, c):
                ps = psum.tile([HC, NB], F32, tag=f"ps1_{c}", name="ps")
                for k in range(NK1):
                    o = L1R_COL + (k * 2 + c) * HC
                    nc.tensor.matmul(
                        ps, wt[0:F32R_K, o : o + HC], xg[:, k, :],
                        start=(k == 0), stop=(k == NK1 - 1),
                    )
                return ps

            def dense_fp8(hh, hl, l, c):
                sz = OUT if l == 4 else HC
                ps = psum.tile([sz, NB], F32, tag=f"ps{l}_{c}", name="ps")
                wfc = wlf8(l, c)
                nc.tensor.matmul(ps, wfc[:, 0:2, :], hh, start=True, stop=False,
                                 perf_mode=DRMODE)
                nc.tensor.matmul(ps, wfc[:, 2:4, :], hh, start=False, stop=False,
                                 perf_mode=DRMODE)
                nc.tensor.matmul(ps, wfc[:, 0:2, :], hl, start=False, stop=True,
                                 perf_mode=DRMODE)
                return ps

            def dense_f32r(hin, l, c):
                col = {2: W2_COL, 3: W3_COL, 4: W4_COL}[l]
                sz = OUT if l == 4 else HC
                ps = psum.tile([sz, NB], F32, tag=f"ps{l}_{c}", name="ps")
                for k in range(2):
                    o = col + ((k * 2 + c) if l < 4 else k) * sz
                    nc.tensor.matmul(
                        ps, wt[0:HC, o : o + sz], hin[k],
                        start=(k == 0), stop=(k == 1),
                    )
                return ps

            # ---------- activation stages ----------
            def act_stage(ps, l, c, t):
                """Emit layer-l chunk-c activation; returns rep handle parts."""
                b_idx = 2 * (l - 1) + c
                scale = OUT_SH[l] / PS_SCALE[l]
                if (l + 1) in fp8l:
                    tt = hpool.tile([HC, NB], F32, tag=f"t{l}_{c}", name="tt")
                    nc.scalar.activation(tt, ps, relu, bias=bias(b_idx, HC),
                                         scale=scale)
                    return tt
                h = hpool.tile([HC, NB], F32R, tag=f"h{l}_{c}", name="h")
                nc.scalar.activation(h, ps, relu, bias=bias(b_idx, HC), scale=scale)
                return h

            def split_stage(tts, l):
                """DVE hi-cast + Pool lo-sub for both chunks -> (hh, hl)."""
                hh = hpool.tile([HC, 2, NB], FP8, tag=f"hh{l}", name="hh")
                hl = hpool.tile([HC, 2, NB], FP8, tag=f"hl{l}", name="hl")
                for c in (0, 1):
                    nc.vector.tensor_scalar_add(hh[:, c, :], tts[c], 0.0)
                    nc.vector.scalar_tensor_tensor(
                        hl[:, c, :], tts[c], 1.0, hh[:, c, :],
                        op0=AOP.mult, op1=AOP.subtract,
                    )
                return hh, hl

            def layer_out(rep, l, t):
                """rep: list of per-chunk ACT products; build next-layer input."""
                if (l + 1) in fp8l:
                    return split_stage(rep, l)
                return rep

            def l4_out(ps, g):
                o = opool.tile([OUT, NB], F32, tag="o", name="o")
                if 4 in fp8l:
                    o2 = opool.tile([OUT, NB], F32, tag="o2", name="o2")
                    nc.scalar.activation(
                        o2, ps, mybir.ActivationFunctionType.Copy,
                        scale=1.0 / PS_SCALE[4],
                    )
                    nc.vector.tensor_scalar_add(o, o2, bias(B4_COL - B_COL, OUT))
                else:
                    nc.vector.tensor_scalar_add(o, ps, bias(B4_COL - B_COL, OUT))
                nc.scalar.dma_start(out=outT.ap()[:, g * NB : (g + 1) * NB], in_=o)

            # ---------- x loads ----------
            def load_x(t):
                g = t % NG
                if 1 in fp8l:
                    xg = xpool.tile([128, NSLOT, NB], FP8, tag="xg", name="xg")
                    src = xh.ap()[g * 128 : (g + 1) * 128, :].rearrange(
                        "p (s b) -> p s b", s=NSLOT)
                    if t == 0:
                        for s in range(NSLOT):
                            nc.sync.dma_start(out=xg[:, s, :], in_=src[:, s, :])
                    else:
                        nc.sync.dma_start(out=xg, in_=src)
                else:
                    xg = xpool.tile([F32R_K, NK1, NB], F32R, tag="xg", name="xg")
                    src = xh.ap()[g * F32R_K : (g + 1) * F32R_K, :].rearrange(
                        "p (k b) -> p k b", k=NK1)
                    if t == 0:
                        for k in range(NK1):
                            nc.sync.dma_start(out=xg[:, k, :], in_=src[:, k, :])
                    else:
                        nc.sync.dma_start(out=xg, in_=src)
                return xg

            def l1(xg, c):
                return l1_fp8(xg, c) if 1 in fp8l else l1_f32r(xg, c)

            def dense(hrep, l, c):
                if l in fp8l:
                    return dense_fp8(hrep[0], hrep[1], l, c)
                return dense_f32r(hrep, l, c)

            # ---------- main loop: 2-stage software-pipeline skew ----------
            # iter t computes L1(t), L3+L4+out(t-2), L2(t-1); activations of
            # group t are split to fp8 a full group before any matmul needs
            # them, so the DVE/ACT chain never stalls the PE.
            # PE order: L1c0(t) | L3(t-2) | L1c1(t) | L4(t-2) | L2(t-1)
            def l3_l4(h2rep, g_out):
                ps30 = dense(h2rep, 3, 0)
                t30 = act_stage(ps30, 3, 0, 0)
                ps31 = dense(h2rep, 3, 1)
                t31 = act_stage(ps31, 3, 1, 0)
                return layer_out([t30, t31], 3, 0)

            def l2_full(h1rep):
                ps20 = dense(h1rep, 2, 0)
                t20 = act_stage(ps20, 2, 0, 0)
                ps21 = dense(h1rep, 2, 1)
                t21 = act_stage(ps21, 2, 1, 0)
                return layer_out([t20, t21], 2, 0)

            h1_s = None  # h1 of t-1
            h2_s = None  # h2 of t-2
            g1 = g2 = None
            for t in range(NG * repeats):
                g = t % NG
                xg = load_x(t)
                ps10 = l1(xg, 0)
                t10 = act_stage(ps10, 1, 0, t)
                h3 = None
                if h2_s is not None:
                    ps30 = dense(h2_s, 3, 0)
                    t30 = act_stage(ps30, 3, 0, t)
                    ps31 = dense(h2_s, 3, 1)
                    t31 = act_stage(ps31, 3, 1, t)
                    h3 = layer_out([t30, t31], 3, t)
                ps11 = l1(xg, 1)
                t11 = act_stage(ps11, 1, 1, t)
                h1 = layer_out([t10, t11], 1, t)
                if h3 is not None:
                    ps4 = dense(h3, 4, 0)
                    l4_out(ps4, g2)
                h2_new = None
                if h1_s is not None:
                    h2_new = l2_full(h1_s)
                h2_s, g2 = h2_new, g1
                h1_s, g1 = h1, g
            # epilogue: drain the two in-flight stages
            if h2_s is not None:
                h3 = l3_l4(h2_s, g2)
                ps4 = dense(h3, 4, 0)
                l4_out(ps4, g2)
            h2_last = l2_full(h1_s)
            h3 = l3_l4(h2_last, g1)
            ps4 = dense(h3, 4, 0)
            l4_out(ps4, g1)

    nc.compile()
    return nc


# ================= host side =================

def _im2col(conv_w: np.ndarray) -> np.ndarray:
    co = IMG - KW + 1
    C = np.zeros((IMG * IMG, co * co), dtype=np.float64)
    ii, jj = np.meshgrid(np.arange(co), np.arange(co), indexing="ij")
    q = (ii * co + jj).ravel()
    for di in range(KW):
        for dj in range(KW):
            p = ((ii + di) * IMG + (jj + dj)).ravel()
            C[p, q] += conv_w[di, dj]
    return C


def _hi_lo(a: np.ndarray, s: float):
    hi = (a * s).astype(E4)
    lo = (a * s - hi.astype(np.float32)).astype(E4)
    assert np.isfinite(hi.astype(np.float32)).all()
    return hi, lo


def _pack_wb(mode, W1p, b1, W2, b2, W3, b3, W4, b4) -> np.ndarray:
    fp8l = FP8L if mode == "fp8" else frozenset()
    OUT_SH, _ = _scales(fp8l)
    wb = np.zeros((128, WC), dtype=np.float32)
    for k in range(NK1):
        for c in range(2):
            wb[0:F32R_K, L1R_COL + (k * 2 + c) * HC : L1R_COL + (k * 2 + c + 1) * HC] = \
                W1p[k * F32R_K : (k + 1) * F32R_K, c * HC : (c + 1) * HC]
    for l, W, col in ((2, W2, W2_COL), (3, W3, W3_COL)):
        for k in range(2):
            for c in range(2):
                wb[0:HC, col + (k * 2 + c) * HC : col + (k * 2 + c + 1) * HC] = \
                    W[k * HC : (k + 1) * HC, c * HC : (c + 1) * HC]
    for k in range(2):
        wb[0:HC, W4_COL + k * OUT : W4_COL + (k + 1) * OUT] = W4[k * HC : (k + 1) * HC]
    for l, b in ((1, b1), (2, b2), (3, b3)):
        for c in range(2):
            wb[0:HC, B_COL + 2 * (l - 1) + c] = b[c * HC : (c + 1) * HC] * OUT_SH[l]
    wb[0:OUT, B4_COL] = b4
    return wb


def _pack_wf(mode, W1p, W2, W3, W4) -> np.ndarray:
    fp8l = FP8L if mode == "fp8" else frozenset()
    F8_OFF, F8C = _f8_offsets(fp8l)
    wf = np.zeros((128, F8C), dtype=E4)
    if 1 in fp8l:
        hi, lo = _hi_lo(W1p, SW[1])
        for c in range(2):
            base = F8_OFF[1] + c * NSLOT * SLOTW
            cw = slice(c * HC, (c + 1) * HC)
            sl = lambda s: slice(base + s * SLOTW, base + s * SLOTW + HC)
            for k in range(XCH):
                wf[:, sl(k)] = hi[k * 128 : (k + 1) * 128, cw]
                wf[:, sl(6 + k)] = lo[k * 128 : (k + 1) * 128, cw]
            # tail slots: 12 = [Wh_tail; Wh_tail], 13 = [Wl_tail; 0]
            wf[0:XTAIL, sl(12)] = hi[XCH * 128 :, cw]
            wf[XTAIL : 2 * XTAIL, sl(12)] = hi[XCH * 128 :, cw]
            wf[0:XTAIL, sl(13)] = lo[XCH * 128 :, cw]
    for l, W in ((2, W2), (3, W3), (4, W4)):
        if l not in fp8l:
            continue
        sz, sw = (OUT, SLOTW4) if l == 4 else (HC, SLOTW)
        hi, lo = _hi_lo(W, SW[l])
        nch = 1 if l == 4 else 2
        for c in range(nch):
            base = F8_OFF[l] + c * 4 * sw
            cw = slice(c * sz, (c + 1) * sz)
            sl = lambda s: slice(base + s * sw, base + s * sw + sz)
            for k in range(2):
                wf[0:HC, sl(k)] = hi[k * HC : (k + 1) * HC, cw]
                wf[0:HC, sl(2 + k)] = lo[k * HC : (k + 1) * HC, cw]
    return wf


def _pack_x_fp8(x_shard: np.ndarray) -> np.ndarray:
    """[8192, 784] f32 -> [NG*128, NSLOT*NB] e4m3 per the slot layout."""
    xT = x_shard.reshape(NG, NB, K1).transpose(0, 2, 1)  # [g, 784, 512]
    hi, lo = _hi_lo(xT, SX)
    out = np.zeros((NG, 128, NSLOT, NB), dtype=E4)
    hif = hi.reshape(NG, K1, NB)
    lof = lo.reshape(NG, K1, NB)
    for k in range(XCH):
        out[:, :, k, :] = hif[:, k * 128 : (k + 1) * 128]
        out[:, :, 6 + k, :] = lof[:, k * 128 : (k + 1) * 128]
    out[:, 0:XTAIL, 12, :] = hif[:, XCH * 128 :]
    out[:, XTAIL : 2 * XTAIL, 12, :] = lof[:, XCH * 128 :]
    out[:, :, 13, :] = out[:, :, 12, :]
    return out.reshape(NG * 128, NSLOT * NB)


def _pack_x_f32r(x_shard: np.ndarray) -> np.ndarray:
    xt = x_shard.T.reshape(NK1, F32R_K, NG, NB)
    return np.ascontiguousarray(xt.transpose(2, 1, 0, 3)).reshape(NG * F32R_K, NK1 * NB)


def prepare_in_maps(mode, x, conv_w, W1, b1, W2, b2, W3, b3, W4, b4):
    x = np.asarray(x, dtype=np.float32)
    C = _im2col(np.asarray(conv_w, dtype=np.float64))
    W1p = (C @ np.asarray(W1, dtype=np.float64)).astype(np.float32)
    f = np.float32
    Ws = [np.asarray(a, f) for a in (W2, W3, W4)]
    bs = [np.asarray(a, f) for a in (b1, b2, b3, b4)]
    wb = _pack_wb(mode, W1p, bs[0], Ws[0], bs[1], Ws[1], bs[2], Ws[2], bs[3])
    wf = _pack_wf(mode, W1p, Ws[0], Ws[1], Ws[2])
    pack_x = _pack_x_fp8 if (mode == "fp8" and 1 in FP8L) else _pack_x_f32r
    in_maps = []
    for c in range(N_CORES):
        in_maps.append({"xh": pack_x(x[c * BC : (c + 1) * BC]), "wb": wb, "wf": wf})
    return in_maps


def kernel(x, conv_w, W1, b1, W2, b2, W3, b3, W4, b4, _mode="fp8"):
    if _mode not in _cache:
        _cache[_mode] = _build(_mode)
    nc = _cache[_mode]
    in_maps = prepare_in_maps(_mode, x, conv_w, W1, b1, W2, b2, W3, b3, W4, b4)
    res = run_bass_kernel_spmd(nc, in_maps, core_ids=list(range(N_CORES)))
    out = np.empty((B, OUT), dtype=np.float32)
    for c in range(N_CORES):
        out[c * BC : (c + 1) * BC] = res.results[c]["outT"].T
    return out


# revision 9
# speedup vs baseline: 1.5383x; 1.5383x over previous
"""Trainium2 Bass kernel for DigitConvolutionalModel (dense_cnn).

Network: x[B,784] -> 3x3 valid conv (1 ch) -> flatten -> MLP
         (676->200 relu, 200->200 relu, 200->200 relu, 200->10).
Conv+W1 fold into W1p = C @ W1 [784, 200] (conv is linear, feeds W1 with no
nonlinearity between), so the network is a plain 4-layer MLP. Sharding: pure
data parallel over 8 cores (batch 65536 -> 8192/core); activations stay
feature-major on device; host packs/unpacks.

Speed scheme (HW-measured 69.0us vs the all-f32r baseline's 80.4us): layer 1
runs its matmuls in fp8 e4m3 with perf_mode=DoubleRow, which contracts TWO
independent (lhsT, rhs) k-tiles per instruction at ~2x the per-column rate
(measured ~111 ns per DR instruction at N=512 vs ~209 ns per f32r matmul -
3.8x f32r throughput per k-tile). fp8's 3 mantissa bits alone would blow the
2e-2 error gate, so L1 accumulates THREE error-compensation terms into one
PSUM group:

    x@W ~= x_hi@W_hi + x_hi@W_lo + x_lo@W_hi,   _hi = e4m3(v*S),
                                                _lo = e4m3(v*S - _hi)

(the dropped x_lo@W_lo term is O(2^-8) relative). x_hi/x_lo are pre-split on
the HOST (free), so no device-side split work exists; HW rel err 1.7e-3.

Why only L1 (FP8L={1} default): fp8 for L2/L3 requires splitting h1/h2 into
hi/lo fp8 on-device. ACT costs (N+352)/1.2 ns and DVE runs 1x at 0.96 GHz
(~690-720 ns per [100,512] pass), so the extra split passes exceed the PE
time they save - measured: FP8L={1,2} 92.6us, {1,2,3} 103.4us, {1} 69.0us.
The f32r layers 2-4 already sit at the PE stream floor (1 cycle/column).

Per-group PE cost at FP8L={1} (measured = theory, zero stalls):
    L1: 20 DR x ~111ns = 2221ns   (f32r would be 14 x 209 = 2926ns)
    L2+L3+L4: 10 f32r x 209ns = 2093ns
    => 4314 ns/group x 16 groups x = 69.0us total; ACT (6 relu instrs,
    4.3us) runs just under the PE time.

All hidden-feature splits are chunks of (100,100) so a DoubleRow pair can
span both chunks at the same SBUF partitions (used when FP8L includes 2/3).
Scales are powers of two folded into the consuming ACT's scale and the
packed biases. A 2-stage software pipeline (iter t computes L1(t),
L3+L4+out(t-2), L2(t-1)) keeps every activation a full group ahead of the
matmul that consumes it.
"""

import numpy as np
import ml_dtypes

import concourse.bacc as bacc
import concourse.mybir as mybir
import concourse.tile as tile
from concourse.bass_utils import run_bass_kernel_spmd

B = 65536
IMG = 28
KW = 3
HID = 200
OUT = 10
K1 = IMG * IMG  # 784

N_CORES = 8
BC = B // N_CORES  # 8192
NB = 512
NG = BC // NB  # 16
HC = 100  # hidden feature chunk (2 chunks of 100)

F32 = mybir.dt.float32
F32R = mybir.dt.float32r
FP8 = mybir.dt.float8e4
E4 = ml_dtypes.float8_e4m3
DRMODE = mybir.MatmulPerfMode.DoubleRow
AOP = mybir.AluOpType

# which layers run fp8-T3 DoubleRow; layer l>=2 in FP8L makes layer l-1 emit
# hi/lo fp8 activations (device-side split via ACT+DVE+Pool).
import os as _os
FP8L = frozenset(int(c) for c in _os.environ.get("BASS_FP8L", "1"))

# fp8 x layout: 6 full 128-row chunks + 16-row tail; slots 0-5 hi, 6-11 lo,
# 12/13 = tail block (rows 0-15 hi-tail, 16-31 lo-tail, rest zero), slot 13
# a copy of 12 so the tail DoubleRow pair has a real second AP step.
XCH = 6
XTAIL = K1 - XCH * 128  # 16
NSLOT = 14
SLOTW = 112   # fp8 weight slot stride (bytes): must be even + 16B-aligned
SLOTW4 = 16   # L4 slot stride

SX = 16.0   # x scale (|x|max ~5.5 -> 88, e4m3 max 240)
SH = 4.0    # fp8 hidden activation scale (h absmax ~18 -> 72)
SW = {1: 32.0, 2: 64.0, 3: 64.0, 4: 64.0}  # weight scales

F32R_K = 112  # f32r L1 k-chunk (7 chunks)
NK1 = K1 // F32R_K

# ---- f32 weight blob column layout ----
L1R_COL = 0                      # 7k x 2c blocks [112, 100]
W2_COL = L1R_COL + NK1 * HID     # 2k x 2c blocks [100, 100]
W3_COL = W2_COL + 2 * HID
W4_COL = W3_COL + 2 * HID        # 2k blocks [100, 10]
B_COL = W4_COL + 2 * OUT         # b1(2 cols) b2(2) b3(2)
B4_COL = B_COL + 6               # b4 [10, 1]
WC = B4_COL + 1

_cache: dict = {}


def _f8_offsets(fp8l):
    off, cols = {}, 0
    if 1 in fp8l:
        off[1] = cols
        cols += 2 * NSLOT * SLOTW
    for l in (2, 3):
        if l in fp8l:
            off[l] = cols
            cols += 2 * 4 * SLOTW
    if 4 in fp8l:
        off[4] = cols
        cols += 4 * SLOTW4
    return off, max(cols, 4)


def _scales(fp8l):
    """per-layer (act_scale, out_sh, ps_scale)."""
    out_sh = {l: (SH if (l + 1) in fp8l else 1.0) for l in (1, 2, 3)}
    ins = {1: SX, 2: out_sh[1], 3: out_sh[2], 4: out_sh[3]}
    ps = {l: (ins[l] * SW[l] if l in fp8l else 1.0) for l in (1, 2, 3, 4)}
    return out_sh, ps


def _build(mode: str = "fp8", repeats: int = 1, xbufs: int = 3, hbufs: int = 2,
           obufs: int = 2):
    fp8l = FP8L if mode == "fp8" else frozenset()
    F8_OFF, F8C = _f8_offsets(fp8l)
    OUT_SH, PS_SCALE = _scales(fp8l)
    relu = mybir.ActivationFunctionType.Relu

    nc = bacc.Bacc("TRN2", target_bir_lowering=False, debug=False)

    if 1 in fp8l:
        xh = nc.dram_tensor("xh", [NG * 128, NSLOT * NB], FP8, kind="ExternalInput")
    else:
        xh = nc.dram_tensor("xh", [NG * F32R_K, NK1 * NB], F32R, kind="ExternalInput")
    wb = nc.dram_tensor("wb", [128, WC], F32R, kind="ExternalInput")
    wf = nc.dram_tensor("wf", [128, F8C], FP8, kind="ExternalInput")
    outT = nc.dram_tensor("outT", [OUT, BC], F32, kind="ExternalOutput")

    with tile.TileContext(nc) as tc:
        with (
            tc.tile_pool(name="wpool", bufs=1) as wpool,
            tc.tile_pool(name="xpool", bufs=xbufs) as xpool,
            tc.tile_pool(name="hpool", bufs=hbufs) as hpool,
            tc.tile_pool(name="opool", bufs=obufs) as opool,
            tc.tile_pool(name="psum", bufs=1, space="PSUM") as psum,
        ):
            wt = wpool.tile([128, WC], F32R, tag="wt")
            nc.scalar.dma_start(out=wt, in_=wb.ap())
            wft = wpool.tile([128, F8C], FP8, tag="wft")
            nc.gpsimd.dma_start(out=wft, in_=wf.ap())

            def bias(idx, hsz):  # f32 bias column [hsz, 1]
                return wt[0:hsz, B_COL + idx : B_COL + idx + 1].bitcast(F32)

            def w1f8(c):  # [128, NSLOT, 100] L1 fp8 slots, M-chunk c
                o = F8_OFF[1] + c * NSLOT * SLOTW
                return wft[:, o : o + NSLOT * SLOTW].rearrange(
                    "p (s m) -> p s m", s=NSLOT)[:, :, 0:HC]

            def wlf8(l, c):  # [100, 4, sz] layer-l fp8 slots (Wh0 Wh1 Wl0 Wl1)
                sz, sw = (OUT, SLOTW4) if l == 4 else (HC, SLOTW)
                o = F8_OFF[l] + c * 4 * sw
                return wft[0:HC, o : o + 4 * sw].rearrange(
                    "p (s m) -> p s m", s=4)[:, :, 0:sz]

            # ---------- matmul chains ----------
            def l1_fp8(xg, c):
                ps = psum.tile([HC, NB], F32, tag=f"ps1_{c}", name="ps")
                wfc = w1f8(c)
                pairs = ((0, 0), (2, 2), (4, 4),     # x_hi @ W_hi
                         (0, 6), (2, 8), (4, 10),    # x_hi @ W_lo
                         (6, 0), (8, 2), (10, 4))    # x_lo @ W_hi
                for j, (xs, ws) in enumerate(pairs):
                    nc.tensor.matmul(
                        ps, wfc[:, ws : ws + 2, :], xg[:, xs : xs + 2, :],
                        start=(j == 0), stop=False, perf_mode=DRMODE,
                    )
                nc.tensor.matmul(  # tails: all three terms in one 32-row pair
                    ps, wfc[0:32, 12:14, :], xg[0:32, 12:14, :],
                    start=False, stop=True, perf_mode=DRMODE,
                )
                return ps

            def l1_f32r(xg, c):
                ps = psum.tile([HC, NB], F32, tag=f"ps1_{c}", name="ps")
                for k in range(NK1):
                    o = L1R_COL + (k * 2 + c) * HC
                    nc.tensor.matmul(
                        ps, wt[0:F32R_K, o : o + HC], xg[:, k, :],
                        start=(k == 0), stop=(k == NK1 - 1),
                    )
                return ps

            def dense_fp8(hh, hl, l, c):
                sz = OUT if l == 4 else HC
                ps = psum.tile([sz, NB], F32, tag=f"ps{l}_{c}", name="ps")
                wfc = wlf8(l, c)
                nc.tensor.matmul(ps, wfc[:, 0:2, :], hh, start=True, stop=False,
                                 perf_mode=DRMODE)
                nc.tensor.matmul(ps, wfc[:, 2:4, :], hh, start=False, stop=False,
                                 perf_mode=DRMODE)
                nc.tensor.matmul(ps, wfc[:, 0:2, :], hl, start=False, stop=True,
                                 perf_mode=DRMODE)
                return ps

            def dense_f32r(hin, l, c):
                col = {2: W2_COL, 3: W3_COL, 4: W4_COL}[l]
                sz = OUT if l == 4 else HC
                ps = psum.tile([sz, NB], F32, tag=f"ps{l}_{c}", name="ps")
                for k in range(2):
                    o = col + ((k * 2 + c) if l < 4 else k) * sz
                    nc.tensor.matmul(
                        ps, wt[0:HC, o : o + sz], hin[k],
                        start=(k == 0), stop=(k == 1),
                    )
                return ps

            # ---------- activation stages ----------
            def act_stage(ps, l, c, t):
                """Emit layer-l chunk-c activation; returns rep handle parts."""
                b_idx = 2 * (l - 1) + c
                scale = OUT_SH[l] / PS_SCALE[l]
                if (l + 1) in fp8l:
                    tt = hpool.tile([HC, NB], F32, tag=f"t{l}_{c}", name="tt")
                    nc.scalar.activation(tt, ps, relu, bias=bias(b_idx, HC),
                                         scale=scale)
                    return tt
                h = hpool.tile([HC, NB], F32R, tag=f"h{l}_{c}", name="h")
                nc.scalar.activation(h, ps, relu, bias=bias(b_idx, HC), scale=scale)
                return h

            def split_stage(tts, l):
                """DVE hi-cast + Pool lo-sub for both chunks -> (hh, hl)."""
                hh = hpool.tile([HC, 2, NB], FP8, tag=f"hh{l}", name="hh")
                hl = hpool.tile([HC, 2, NB], FP8, tag=f"hl{l}", name="hl")
                for c in (0, 1):
                    nc.vector.tensor_scalar_add(hh[:, c, :], tts[c], 0.0)
                    nc.vector.scalar_tensor_tensor(
                        hl[:, c, :], tts[c], 1.0, hh[:, c, :],
                        op0=AOP.mult, op1=AOP.subtract,
                    )
                return hh, hl

            def layer_out(rep, l, t):
                """rep: list of per-chunk ACT products; build next-layer input."""
                if (l + 1) in fp8l:
                    return split_stage(rep, l)
                return rep

            def l4_out(ps, g):
                o = opool.tile([OUT, NB], F32, tag="o", name="o")
                if 4 in fp8l:
                    o2 = opool.tile([OUT, NB], F32, tag="o2", name="o2")
                    nc.scalar.activation(
                        o2, ps, mybir.ActivationFunctionType.Copy,
                        scale=1.0 / PS_SCALE[4],
                    )
                    nc.vector.tensor_scalar_add(o, o2, bias(B4_COL - B_COL, OUT))
                else:
                    nc.vector.tensor_scalar_add(o, ps, bias(B4_COL - B_COL, OUT))
                nc.scalar.dma_start(out=outT.ap()[:, g * NB : (g + 1) * NB], in_=o)

            # ---------- x loads ----------
            def load_x(t):
                g = t % NG
                if 1 in fp8l:
                    xg = xpool.tile([128, NSLOT, NB], FP8, tag="xg", name="xg")
                    src = xh.ap()[g * 128 : (g + 1) * 128, :].rearrange(
                        "p (s b) -> p s b", s=NSLOT)
                    if t == 0:
                        for s in range(NSLOT):
                            nc.sync.dma_start(out=xg[:, s, :], in_=src[:, s, :])
                    else:
                        nc.sync.dma_start(out=xg, in_=src)
                else:
                    xg = xpool.tile([F32R_K, NK1, NB], F32R, tag="xg", name="xg")
                    src = xh.ap()[g * F32R_K : (g + 1) * F32R_K, :].rearrange(
                        "p (k b) -> p k b", k=NK1)
                    if t == 0:
                        for k in range(NK1):
                            nc.sync.dma_start(out=xg[:, k, :], in_=src[:, k, :])
                    else:
                        nc.sync.dma_start(out=xg, in_=src)
                return xg

            def l1(xg, c):
                return l1_fp8(xg, c) if 1 in fp8l else l1_f32r(xg, c)

            def dense(hrep, l, c):
                if l in fp8l:
                    return dense_fp8(hrep[0], hrep[1], l, c)
                return dense_f32r(hrep, l, c)

            # ---------- main loop: 2-stage software-pipeline skew ----------
            # iter t computes L1(t), L3+L4+out(t-2), L2(t-1); activations of
            # group t are split to fp8 a full group before any matmul needs
            # them, so the DVE/ACT chain never stalls the PE.
            # PE order: L1c0(t) | L3(t-2) | L1c1(t) | L4(t-2) | L2(t-1)
            def l3_l4(h2rep, g_out):
                ps30 = dense(h2rep, 3, 0)
                t30 = act_stage(ps30, 3, 0, 0)
                ps31 = dense(h2rep, 3, 1)
                t31 = act_stage(ps31, 3, 1, 0)
                return layer_out([t30, t31], 3, 0)

            def l2_full(h1rep):
                ps20 = dense(h1rep, 2, 0)
                t20 = act_stage(ps20, 2, 0, 0)
                ps21 = dense(h1rep, 2, 1)
                t21 = act_stage(ps21, 2, 1, 0)
                return layer_out([t20, t21], 2, 0)

            h1_s = None  # h1 of t-1
            h2_s = None  # h2 of t-2
            g1 = g2 = None
            for t in range(NG * repeats):
                g = t % NG
                xg = load_x(t)
                ps10 = l1(xg, 0)
                t10 = act_stage(ps10, 1, 0, t)
                h3 = None
                if h2_s is not None:
                    ps30 = dense(h2_s, 3, 0)
                    t30 = act_stage(ps30, 3, 0, t)
                    ps31 = dense(h2_s, 3, 1)
                    t31 = act_stage(ps31, 3, 1, t)
                    h3 = layer_out([t30, t31], 3, t)
                ps11 = l1(xg, 1)
                t11 = act_stage(ps11, 1, 1, t)
                h1 = layer_out([t10, t11], 1, t)
                if h3 is not None:
                    ps4 = dense(h3, 4, 0)
                    l4_out(ps4, g2)
                h2_new = None
                if h1_s is not None:
                    h2_new = l2_full(h1_s)
                h2_s, g2 = h2_new, g1
                h1_s, g1 = h1, g
            # epilogue: drain the two in-flight stages
            if h2_s is not None:
                h3 = l3_l4(h2_s, g2)
                ps4 = dense(h3, 4, 0)
                l4_out(ps4, g2)
            h2_last = l2_full(h1_s)
            h3 = l3_l4(h2_last, g1)
            ps4 = dense(h3, 4, 0)
            l4_out(ps4, g1)

    nc.compile()
    return nc


# ================= host side =================

def _im2col(conv_w: np.ndarray) -> np.ndarray:
    co = IMG - KW + 1
    C = np.zeros((IMG * IMG, co * co), dtype=np.float64)
    ii, jj = np.meshgrid(np.arange(co), np.arange(co), indexing="ij")
    q = (ii * co + jj).ravel()
    for di in range(KW):
        for dj in range(KW):
            p = ((ii + di) * IMG + (jj + dj)).ravel()
            C[p, q] += conv_w[di, dj]
    return C


def _hi_lo(a: np.ndarray, s: float):
    hi = (a * s).astype(E4)
    lo = (a * s - hi.astype(np.float32)).astype(E4)
    assert np.isfinite(hi.astype(np.float32)).all()
    return hi, lo


def _pack_wb(mode, W1p, b1, W2, b2, W3, b3, W4, b4) -> np.ndarray:
    fp8l = FP8L if mode == "fp8" else frozenset()
    OUT_SH, _ = _scales(fp8l)
    wb = np.zeros((128, WC), dtype=np.float32)
    for k in range(NK1):
        for c in range(2):
            wb[0:F32R_K, L1R_COL + (k * 2 + c) * HC : L1R_COL + (k * 2 + c + 1) * HC] = \
                W1p[k * F32R_K : (k + 1) * F32R_K, c * HC : (c + 1) * HC]
    for l, W, col in ((2, W2, W2_COL), (3, W3, W3_COL)):
        for k in range(2):
            for c in range(2):
                wb[0:HC, col + (k * 2 + c) * HC : col + (k * 2 + c + 1) * HC] = \
                    W[k * HC : (k + 1) * HC, c * HC : (c + 1) * HC]
    for k in range(2):
        wb[0:HC, W4_COL + k * OUT : W4_COL + (k + 1) * OUT] = W4[k * HC : (k + 1) * HC]
    for l, b in ((1, b1), (2, b2), (3, b3)):
        for c in range(2):
            wb[0:HC, B_COL + 2 * (l - 1) + c] = b[c * HC : (c + 1) * HC] * OUT_SH[l]
    wb[0:OUT, B4_COL] = b4
    return wb


def _pack_wf(mode, W1p, W2, W3, W4) -> np.ndarray:
    fp8l = FP8L if mode == "fp8" else frozenset()
    F8_OFF, F8C = _f8_offsets(fp8l)
    wf = np.zeros((128, F8C), dtype=E4)
    if 1 in fp8l:
        hi, lo = _hi_lo(W1p, SW[1])
        for c in range(2):
            base = F8_OFF[1] + c * NSLOT * SLOTW
            cw = slice(c * HC, (c + 1) * HC)
            sl = lambda s: slice(base + s * SLOTW, base + s * SLOTW + HC)
            for k in range(XCH):
                wf[:, sl(k)] = hi[k * 128 : (k + 1) * 128, cw]
                wf[:, sl(6 + k)] = lo[k * 128 : (k + 1) * 128, cw]
            # tail slots: 12 = [Wh_tail; Wh_tail], 13 = [Wl_tail; 0]
            wf[0:XTAIL, sl(12)] = hi[XCH * 128 :, cw]
            wf[XTAIL : 2 * XTAIL, sl(12)] = hi[XCH * 128 :, cw]
            wf[0:XTAIL, sl(13)] = lo[XCH * 128 :, cw]
    for l, W in ((2, W2), (3, W3), (4, W4)):
        if l not in fp8l:
            continue
        sz, sw = (OUT, SLOTW4) if l == 4 else (HC, SLOTW)
        hi, lo = _hi_lo(W, SW[l])
        nch = 1 if l == 4 else 2
        for c in range(nch):
            base = F8_OFF[l] + c * 4 * sw
            cw = slice(c * sz, (c + 1) * sz)
            sl = lambda s: slice(base + s * sw, base + s * sw + sz)
            for k in range(2):
                wf[0:HC, sl(k)] = hi[k * HC : (k + 1) * HC, cw]
                wf[0:HC, sl(2 + k)] = lo[k * HC : (k + 1) * HC, cw]
    return wf


def _pack_x_fp8(x_shard: np.ndarray) -> np.ndarray:
    """[8192, 784] f32 -> [NG*128, NSLOT*NB] e4m3 per the slot layout."""
    xT = x_shard.reshape(NG, NB, K1).transpose(0, 2, 1)  # [g, 784, 512]
    hi, lo = _hi_lo(xT, SX)
    out = np.zeros((NG, 128, NSLOT, NB), dtype=E4)
    hif = hi.reshape(NG, K1, NB)
    lof = lo.reshape(NG, K1, NB)
    for k in range(XCH):
        out[:, :, k, :] = hif[:, k * 128 : (k + 1) * 128]
        out[:, :, 6 + k, :] = lof[:, k * 128 : (k + 1) * 128]
    out[:, 0:XTAIL, 12, :] = hif[:, XCH * 128 :]
    out[:, XTAIL : 2 * XTAIL, 12, :] = lof[:, XCH * 128 :]
    out[:, :, 13, :] = out[:, :, 12, :]
    return out.reshape(NG * 128, NSLOT * NB)


def _pack_x_f32r(x_shard: np.ndarray) -> np.ndarray:
    xt = x_shard.T.reshape(NK1, F32R_K, NG, NB)
    return np.ascontiguousarray(xt.transpose(2, 1, 0, 3)).reshape(NG * F32R_K, NK1 * NB)


def prepare_in_maps(mode, x, conv_w, W1, b1, W2, b2, W3, b3, W4, b4):
    x = np.asarray(x, dtype=np.float32)
    C = _im2col(np.asarray(conv_w, dtype=np.float64))
    W1p = (C @ np.asarray(W1, dtype=np.float64)).astype(np.float32)
    f = np.float32
    Ws = [np.asarray(a, f) for a in (W2, W3, W4)]
    bs = [np.asarray(a, f) for a in (b1, b2, b3, b4)]
    wb = _pack_wb(mode, W1p, bs[0], Ws[0], bs[1], Ws[1], bs[2], Ws[2], bs[3])
    wf = _pack_wf(mode, W1p, Ws[0], Ws[1], Ws[2])
    pack_x = _pack_x_fp8 if (mode == "fp8" and 1 in FP8L) else _pack_x_f32r
    in_maps = []
    for c in range(N_CORES):
        in_maps.append({"xh": pack_x(x[c * BC : (c + 1) * BC]), "wb": wb, "wf": wf})
    return in_maps


def kernel(x, conv_w, W1, b1, W2, b2, W3, b3, W4, b4, _mode="fp8"):
    if _mode not in _cache:
        _cache[_mode] = _build(_mode)
    nc = _cache[_mode]
    in_maps = prepare_in_maps(_mode, x, conv_w, W1, b1, W2, b2, W3, b3, W4, b4)
    res = run_bass_kernel_spmd(nc, in_maps, core_ids=list(range(N_CORES)))
    out = np.empty((B, OUT), dtype=np.float32)
    for c in range(N_CORES):
        out[c * BC : (c + 1) * BC] = res.results[c]["outT"].T
    return out
